# revision 16
# baseline (speedup 1.0000x reference)
"""Trainium2 Bass kernel for nn_D2RLCritic (gnn_message_passing).

Self-contained: kernel(**inputs) takes the FULL unsharded inputs (as from
setup_inputs()) and returns the FULL [256, 1] output, running an SPMD Bass
kernel across 8 NeuronCores.

Design: dst-sharded graph (12544 nodes/core, 98 blocks of 128). Per-edge
feature fetches use gpsimd ap_gather from SBUF-resident transposed feature
stripes ([128, 12544]: partition 16r+f = feature f of node range r), with
per-16-partition-group index streams. L1 projects x@w1l per node first, so
both layers gather 16-dim rows. Gathered columns are transposed on PE into
slot-major E tiles; a bf16 one-hot (dst within block) matmul accumulates the
segment sum in PSUM. Degrees/masks are host-precomputed index tables.
"""

import numpy as np
from contextlib import ExitStack

from concourse import bass, bacc, mybir, tile
from concourse.mybir import AluOpType as ALU
from concourse.mybir import ActivationFunctionType as AF

P = 128
NR = 8
dt = mybir.dt
EPS = 1e-5
CW = 4096


def build_host_data(x, edge_index, batch, n_cores, G):
    assert n_cores == NR
    x = np.ascontiguousarray(np.asarray(x, np.float32))
    src_g = np.asarray(edge_index[0], np.int64)
    dst_g = np.asarray(edge_index[1], np.int64)
    batch = np.asarray(batch, np.int64)
    N0, F = x.shape
    NS = ((N0 + NR * P - 1) // (NR * P)) * P  # 12544
    Npad = NS * NR
    NB = NS // P

    xp = np.zeros((Npad, F), np.float32)
    xp[:N0] = x
    deg = np.bincount(dst_g, minlength=Npad).astype(np.int64)
    batchp = np.full(Npad, -1, np.int64)
    batchp[:N0] = batch

    # per-core edge sort and cell counts
    per_s, per_d, per_cnt = [], [], []
    for k in range(NR):
        m = (dst_g >= k * NS) & (dst_g < (k + 1) * NS)
        s = src_g[m]
        d = dst_g[m] - k * NS
        blk = d >> 7
        rng = s // NS
        order = np.lexsort((s, rng, blk))
        s, d, blk, rng = s[order], d[order], blk[order], rng[order]
        cnt = np.zeros((NB, NR), np.int64)
        np.add.at(cnt, (blk, rng), 1)
        per_s.append(s)
        per_d.append(d)
        per_cnt.append(cnt)
    cnts = np.stack(per_cnt)            # [NR_cores, NB, NR]
    W = cnts.max(axis=0)                # [NB, NR]

    S = np.zeros((NB, NR), np.int64)    # stream offset of cell (b, r)
    S[1:] = np.cumsum(W, axis=0)[:-1]
    L_r = S[-1] + W[-1]
    L = int(((L_r.max() + P - 1) // P) * P)
    NCH = (L + CW - 1) // CW

    # pieces: per block, ordered list of (r, window, drel_col); windows are
    # 128-col spans of all 8 streams (one transposed square serves 8 ranges)
    pieces = []
    npiece = 0
    for b in range(NB):
        plist = []
        for r in range(NR):
            a, z = int(S[b, r]), int(S[b, r] + W[b, r])
            for win in range(a // P, (z + P - 1) // P):
                plist.append((r, win, npiece))
                npiece += 1
        plist.sort(key=lambda t: (t[1], t[0]))
        pieces.append(plist)
    NPIECE = npiece
    in_maps = []
    for k in range(NR):
        s, d, cnt = per_s[k], per_d[k], per_cnt[k]
        # cell start offsets in the sorted edge array
        estart = np.concatenate([[0], np.cumsum(cnt.ravel())])[:-1].reshape(NB, NR)
        # gather index streams, wrapped per 16-partition group
        apg = np.zeros((P, L // 16), np.int16)
        dstrel = np.full((NR, L), -1, np.int64)
        for r in range(NR):
            stream = np.zeros(L, np.int64)
            for b in range(NB):
                n_e = int(cnt[b, r])
                if n_e:
                    e0 = estart[b, r]
                    stream[S[b, r]: S[b, r] + n_e] = s[e0: e0 + n_e] - r * NS
                    dstrel[r, S[b, r]: S[b, r] + n_e] = d[e0: e0 + n_e] - b * P
            apg[16 * r: 16 * (r + 1), :] = (
                stream.reshape(L // 16, 16).T.astype(np.int16)
            )
        # mtbig: [128, NPIECE*128] bf16 inverse-degree-scaled one-hots:
        # mt[p, pc*128+dd] = 1/deg(dst) if slot p of piece pc maps to block
        # offset dd, else 0
        import ml_dtypes
        drel = np.full((P, NPIECE), -1, np.int64)
        pblk = np.zeros(NPIECE, np.int64)
        for b in range(NB):
            for (r, win, pc) in pieces[b]:
                a, z = int(S[b, r]), int(S[b, r] + W[b, r])
                lo, hi = win * P, (win + 1) * P
                aa, zz = max(a, lo), min(z, hi)
                col = np.full(P, -1, np.int64)
                col[aa - lo: zz - lo] = dstrel[r, aa: zz]
                drel[:, pc] = col
                pblk[pc] = b
        onehot = (drel[:, :, None] == np.arange(P)[None, None, :])
        nodes_all = np.arange(NS) + k * NS
        invd_full = (1.0 / np.maximum(deg[nodes_all], 1)).astype(np.float32)
        inv_pc = invd_full.reshape(NB, P)[pblk]          # [NPIECE, 128]
        mtbig = (onehot * inv_pc[None, :, :]).astype(ml_dtypes.bfloat16)
        mtbig = mtbig.reshape(P, NPIECE * P)
        nodes = np.arange(NS) + k * NS
        grel = np.where(nodes < N0, batchp[nodes], -1).astype(np.float32)
        in_maps.append(dict(
            xown=xp[k * NS:(k + 1) * NS],
            apgidx=apg,
            mtbig=mtbig,
            grel=grel.reshape(NB, P).T.copy(),
        ))

    cfg = dict(N=N0, NS=NS, NB=NB, F=F, G=G, NPIECE=NPIECE, L=L, NCH=NCH,
               pieces=pieces, n_cores=NR)
    return in_maps, cfg


def add_weights(in_maps, inputs):
    f32 = np.float32
    w = {}
    w["w1cat"] = np.concatenate(
        [np.asarray(inputs["w1l"], f32), np.asarray(inputs["w1r"], f32)], axis=1
    )  # [64, 32]
    w["w2l"] = np.asarray(inputs["w2l"], f32)
    w["w2r"] = np.asarray(inputs["w2r"], f32)
    for name in ("b1l", "b2l", "g1", "be1"):
        w[name] = np.asarray(inputs[name], f32).reshape(16, 1)
    for name in ("gl1", "bl1", "bW1", "bW2", "bW3"):
        w[name] = np.asarray(inputs[name], f32).reshape(16, 1)
    w["bWf"] = np.asarray(inputs["bWf"], f32).reshape(1, 1)
    for name in ("gl2", "bl2", "gl3", "bl3"):
        v = np.asarray(inputs[name], f32).reshape(32, 1)
        w[name + "a"], w[name + "b"] = v[:16].copy(), v[16:].copy()
    w["W1"] = np.asarray(inputs["W1"], f32)
    w["Wf"] = np.asarray(inputs["Wf"], f32)
    for name in ("W2", "W3"):
        v = np.asarray(inputs[name], f32)
        w[name + "a"], w[name + "b"] = v[:16].copy(), v[16:].copy()
    for m in in_maps:
        m.update(w)
    return in_maps


def build_program(cfg, enable_asserts=False):
    NCORES = cfg["n_cores"]
    N, NS, NB, F, G = cfg["N"], cfg["NS"], cfg["NB"], cfg["F"], cfg["G"]
    NPIECE, L, NCH = cfg["NPIECE"], cfg["L"], cfg["NCH"]
    pieces = cfg["pieces"]
    GT = (G + P - 1) // P
    f32, bf16 = dt.float32, dt.bfloat16

    nc = bacc.Bacc(
        "TRN2", target_bir_lowering=False, debug=False,
        enable_asserts=enable_asserts, num_devices=NCORES,
    )
    RG = [list(range(NCORES))]

    xown_in = nc.dram_tensor("xown", [NS, F], f32, kind="ExternalInput")
    apg_in = nc.dram_tensor("apgidx", [P, L // 16], dt.int16, kind="ExternalInput")
    mtbig_in = nc.dram_tensor("mtbig", [P, NPIECE * P], dt.bfloat16, kind="ExternalInput")
    grel_in = nc.dram_tensor("grel", [P, NB], f32, kind="ExternalInput")
    w1cat_in = nc.dram_tensor("w1cat", [F, 32], f32, kind="ExternalInput")
    w2l_in = nc.dram_tensor("w2l", [16, 16], f32, kind="ExternalInput")
    w2r_in = nc.dram_tensor("w2r", [16, 16], f32, kind="ExternalInput")
    row_ins = {
        name: nc.dram_tensor(name, [16, 1], f32, kind="ExternalInput")
        for name in ("b1l", "b2l", "g1", "be1")
    }
    col_names = ("gl1", "bl1", "bW1", "gl2a", "gl2b", "bl2a", "bl2b",
                 "gl3a", "gl3b", "bl3a", "bl3b", "bW2", "bW3")
    col_ins = {
        name: nc.dram_tensor(name, [16, 1], f32, kind="ExternalInput")
        for name in col_names
    }
    col_ins["bWf"] = nc.dram_tensor("bWf", [1, 1], f32, kind="ExternalInput")
    W_ins = {
        name: nc.dram_tensor(name, [16, shp1], f32, kind="ExternalInput")
        for name, shp1 in (
            ("W1", 16), ("W2a", 16), ("W2b", 16), ("W3a", 16), ("W3b", 16), ("Wf", 1),
        )
    }
    out_t = nc.dram_tensor("out", [1, G], f32, kind="ExternalOutput")
    dbg = cfg.get("debug")
    if dbg:
        dbgy = nc.dram_tensor("dbgy", [P, NS], f32, kind="ExternalOutput")
        dbgh1 = nc.dram_tensor("dbgh1", [P, NS], f32, kind="ExternalOutput")
        dbgxe = nc.dram_tensor("dbgxe", [G, 17], f32, kind="ExternalOutput")


    y1tsh = nc.dram_tensor("y1tsh", [16, NS], f32, kind="Internal")
    y1tall = nc.dram_tensor("y1tall", [P, NS], f32, kind="Internal", addr_space="Shared")
    h1tsh = nc.dram_tensor("h1tsh", [16, NS], f32, kind="Internal")
    h1tall = nc.dram_tensor("h1tall", [P, NS], f32, kind="Internal", addr_space="Shared")
    stin = nc.dram_tensor("stin", [16, 2], f32, kind="Internal")
    stout = nc.dram_tensor("stout", [16, 2], f32, kind="Internal", addr_space="Shared")
    xein = nc.dram_tensor("xein", [G, 17], f32, kind="Internal")
    xeout = nc.dram_tensor("xeout", [G, 17], f32, kind="Internal", addr_space="Shared")

    iota128_t = nc.inline_tensor(
        np.broadcast_to(np.arange(P, dtype=np.float32), (P, P)).copy(), "iota128"
    )
    iotag_t = nc.inline_tensor(
        np.broadcast_to(np.arange(G, dtype=np.float32), (P, G)).copy(), "iotag"
    )
    ident_t = nc.inline_tensor(np.eye(P, dtype=np.float32), "ident")

    with tile.TileContext(nc) as tc, ExitStack() as top:
        persist = top.enter_context(tc.tile_pool(name="persist", bufs=1))

        iota_f = persist.tile([P, P], f32)
        nc.sync.dma_start(out=iota_f[:], in_=iota128_t.ap())
        iotag_s = persist.tile([P, G], f32)
        nc.sync.dma_start(out=iotag_s[:], in_=iotag_t.ap())
        ident_s = persist.tile([P, P], f32)
        nc.sync.dma_start(out=ident_s[:], in_=ident_t.ap())
        apg_s = persist.tile([P, L // 16], dt.int16)
        nc.sync.dma_start(out=apg_s[:], in_=apg_in.ap())
        grel_s = persist.tile([P, NB], f32)
        nc.sync.dma_start(out=grel_s[:], in_=grel_in.ap())
        w1cat_s = persist.tile([F, 32], f32)
        nc.sync.dma_start(out=w1cat_s[:], in_=w1cat_in.ap())
        w2l_s = persist.tile([16, 16], f32)
        nc.sync.dma_start(out=w2l_s[:], in_=w2l_in.ap())
        w2r_s = persist.tile([16, 16], f32)
        nc.sync.dma_start(out=w2r_s[:], in_=w2r_in.ap())
        rows_s = {}
        for name, t in row_ins.items():
            rows_s[name] = persist.tile([16, 1], f32, tag=f"row_{name}", name=f"row_{name}")
            nc.sync.dma_start(out=rows_s[name][:], in_=t.ap())
        cols_s = {}
        for name, t in col_ins.items():
            cols_s[name] = persist.tile(list(t.shape), f32, tag=f"col_{name}", name=f"col_{name}")
            nc.sync.dma_start(out=cols_s[name][:], in_=t.ap())
        Ws_s = {}
        for name, t in W_ins.items():
            Ws_s[name] = persist.tile(list(t.shape), f32, tag=f"W_{name}", name=f"W_{name}")
            nc.sync.dma_start(out=Ws_s[name][:], in_=t.ap())

        w1cat_b = persist.tile([F, 32], bf16)
        nc.vector.tensor_copy(out=w1cat_b[:], in_=w1cat_s[:])
        w2l_b = persist.tile([16, 16], bf16)
        nc.vector.tensor_copy(out=w2l_b[:], in_=w2l_s[:])
        w2r_b = persist.tile([16, 16], bf16)
        nc.vector.tensor_copy(out=w2r_b[:], in_=w2r_s[:])
        ident16_b = persist.tile([16, 16], bf16)
        nc.vector.tensor_copy(out=ident16_b[:], in_=ident_s[:16, :16])

        stripe_s = persist.tile([P, NS], f32)        # y1T then h1T (gathered)
        ytown = persist.tile([16, NS], f32)          # y1T own, then h1T own
        xrbt_own = persist.tile([16, NS], bf16)      # (x @ w1r + b1l)^T own
        statacc = persist.tile([16, 2], f32)
        nc.vector.memset(statacc[:], 0.0)
        a_col = persist.tile([16, 1], f32, tag="a_col")
        c_col = persist.tile([16, 1], f32, tag="c_col")

        # warmup: load ap_gather ucode early
        with tc.tile_pool(name="warm", bufs=1) as wp:
            wi = wp.tile([P, 16], dt.int16)
            nc.gpsimd.memset(wi[:], 0)
            wo = wp.tile([P, 16], f32)
            nc.gpsimd.ap_gather(
                out_ap=wo[:], in_ap=iota_f[:], idxs_ap=wi[:, 0:1],
                channels=P, num_elems=P, d=1, num_idxs=16,
            )

        # ================= L1 prep: y1T own + xrbT =================
        with tc.tile_pool(name="p1", bufs=3) as pl, tc.tile_pool(
            name="p1ps", bufs=2, space="PSUM"
        ) as ps1, tc.tile_pool(name="p1s", bufs=3) as sb1:
            XB = 4
            xbig = {}
            for b in range(NB):
                g = b // XB
                if g not in xbig:
                    nbl = min(XB, NB - g * XB)
                    xt = pl.tile([P, XB * F], f32, tag="xb")
                    nc.sync.dma_start(
                        out=xt[:, 0:nbl * F].rearrange("p (a f) -> p a f", a=nbl),
                        in_=xown_in.ap()[g * XB * P:(g * XB + nbl) * P, :].rearrange(
                            "(a p) f -> p a f", p=P),
                    )
                    xbig[g] = xt
                xb = xbig[g]
                xTp = ps1.tile([F, P], f32, tag="xTp", name="xTp")
                nc.tensor.transpose(out=xTp[:], in_=xb[:, (b % XB) * F:(b % XB + 1) * F],
                                    identity=ident_s[:])
                xT_s = sb1.tile([F, P], bf16, tag="xTs")
                nc.vector.tensor_copy(out=xT_s[:], in_=xTp[:])
                y1p = ps1.tile([16, P], f32, tag="y1p", name="y1p")
                nc.tensor.matmul(out=y1p[:], lhsT=w1cat_b[:, 0:16], rhs=xT_s[:],
                                 start=True, stop=True)
                nc.vector.tensor_copy(out=ytown[:, b * P:(b + 1) * P], in_=y1p[:])
                xrp = ps1.tile([16, P], f32, tag="xrp", name="xrp")
                nc.tensor.matmul(out=xrp[:], lhsT=w1cat_b[:, 16:32], rhs=xT_s[:],
                                 start=True, stop=True)
                nc.vector.tensor_scalar(
                    out=xrbt_own[:, b * P:(b + 1) * P], in0=xrp[:],
                    scalar1=rows_s["b1l"][:], scalar2=None, op0=ALU.add,
                )

        nc.sync.dma_start(out=y1tsh.ap(), in_=ytown[:])
        nc.gpsimd.collective_compute(
            "AllGather", ALU.bypass, replica_groups=RG,
            ins=[y1tsh.ap()], outs=[y1tall.ap()],
        )
        for j in range(8):
            eng = nc.sync if j % 2 == 0 else nc.scalar
            eng.dma_start(out=stripe_s[16 * j:16 * (j + 1), :],
                          in_=y1tall.ap()[16 * j:16 * (j + 1), :])
        if dbg:
            nc.sync.dma_start(out=dbgy.ap(), in_=y1tall.ap())

        # ================= shared edge-layer emitter =================
        WPC = CW // P   # windows per chunk
        MTB = 16        # one-hot pieces per DMA batch
        fence_pool = top.enter_context(tc.tile_pool(name="fence", bufs=2))

        def pool_fence():
            """Order later gpsimd work after the stripe/idx loads: gpsimd is
            in-order, and this op's reads are dependency-tracked."""
            fp = fence_pool.tile([1, 4], f32, tag="fence", name="fence")
            nc.gpsimd.tensor_tensor(out=fp[:], in0=stripe_s[0:1, 0:4],
                                    in1=apg_s[0:1, 0:8].bitcast(f32),
                                    op=ALU.add)

        def emit_layer(layer, epilogue, start_extra):
            """Gather + window transposes + psdT accumulation per block.

            psdT[f, d] = sum over pieces of E_piece^T @ MTinv_piece; the
            host-prepared MTinv folds 1/deg. start_extra(b, psd) may emit an
            initial accumulating matmul (returns True if it started the
            group)."""
            pool_fence()
            with tc.tile_pool(name=f"ch{layer}", bufs=3) as chp, tc.tile_pool(
                name=f"sqps{layer}", bufs=2, space="PSUM"
            ) as sqps, tc.tile_pool(name=f"sq{layer}", bufs=8) as sqp, tc.tile_pool(
                name=f"mt{layer}", bufs=3
            ) as mtp, tc.tile_pool(name=f"psd{layer}", bufs=2, space="PSUM") as psdp, \
                 tc.tile_pool(name=f"ep{layer}", bufs=3) as epp, tc.tile_pool(
                name=f"epps{layer}", bufs=1, space="PSUM"
            ) as epps:
                chunks = {}
                squares = {}
                mtts = {}
                next_ch = 0

                def ensure_window(win):
                    nonlocal next_ch
                    if win in squares:
                        return
                    while next_ch <= win // WPC and next_ch < NCH:
                        cw = min(CW, L - next_ch * CW)
                        ct = chp.tile([P, CW], f32, tag="chunk")
                        nc.gpsimd.ap_gather(
                            out_ap=ct[:, 0:cw], in_ap=stripe_s[:],
                            idxs_ap=apg_s[:, next_ch * (CW // 16):
                                          next_ch * (CW // 16) + cw // 16],
                            channels=P, num_elems=NS, d=1, num_idxs=cw,
                        )
                        chunks[next_ch] = ct
                        next_ch += 1
                    cc = (win % WPC) * P
                    sq_ps = sqps.tile([P, P], f32, tag="sqps", name="sqps")
                    nc.tensor.transpose(
                        out=sq_ps[:], in_=chunks[win // WPC][:, cc:cc + P],
                        identity=ident_s[:],
                    )
                    sq = sqp.tile([P, P], bf16, tag="sq")
                    nc.scalar.activation(out=sq[:], in_=sq_ps[:], func=AF.Copy)
                    squares[win] = sq

                def ensure_mt(pc):
                    g = pc // MTB
                    if g not in mtts:
                        g0 = g * MTB * P
                        gw = min(MTB * P, NPIECE * P - g0)
                        mt = mtp.tile([P, MTB * P], bf16, tag="mtb")
                        nc.sync.dma_start(out=mt[:, 0:gw],
                                          in_=mtbig_in.ap()[:, g0:g0 + gw])
                        mtts[g] = mt
                    return mtts[g], (pc % MTB) * P

                for b in range(NB):
                    for (r, win, pc) in pieces[b]:
                        ensure_window(win)
                        ensure_mt(pc)
                    psd = psdp.tile([16, P], f32, tag="psd", name="psd")
                    started = start_extra(b, psd)
                    np_b = len(pieces[b])
                    for i, (r, win, pc) in enumerate(pieces[b]):
                        mt, mo = ensure_mt(pc)
                        nc.tensor.matmul(
                            out=psd[:], lhsT=squares[win][:, 16 * r: 16 * (r + 1)],
                            rhs=mt[:, mo: mo + P],
                            start=(i == 0 and not started), stop=(i == np_b - 1),
                            skip_group_check=True,
                        )
                    epilogue(b, psd, epp, epps)

        # ================= L1 main =================
        def l1_start(b, psd):
            nc.tensor.matmul(
                out=psd[:], lhsT=ident16_b[:],
                rhs=xrbt_own[:, b * P:(b + 1) * P],
                start=True, stop=False, skip_group_check=True,
            )
            return True

        def l1_epilogue(b, psd, epp, epps):
            h1t = epp.tile([16, P], f32, tag="h1t", name="h1t")
            nc.scalar.activation(out=h1t[:], in_=psd[:], func=AF.Relu)
            nc.vector.tensor_copy(out=ytown[:, b * P:(b + 1) * P], in_=h1t[:])
            nc.sync.dma_start(out=h1tsh.ap()[:, b * P:(b + 1) * P], in_=h1t[:])
            sq1 = epp.tile([16, P], f32, tag="sq1", name="sq1")
            nc.scalar.square(out=sq1[:], in_=h1t[:])
            red = epp.tile([16, 2], f32, tag="red", name="red")
            nc.vector.tensor_reduce(out=red[:, 0:1], in_=h1t[:],
                                    axis=mybir.AxisListType.X, op=ALU.add)
            nc.vector.tensor_reduce(out=red[:, 1:2], in_=sq1[:],
                                    axis=mybir.AxisListType.X, op=ALU.add)
            nc.vector.tensor_tensor(out=statacc[:], in0=statacc[:], in1=red[:],
                                    op=ALU.add)

        emit_layer(1, l1_epilogue, l1_start)

        nc.gpsimd.collective_compute(
            "AllGather", ALU.bypass, replica_groups=RG,
            ins=[h1tsh.ap()], outs=[h1tall.ap()],
        )
        with tc.tile_pool(name="st", bufs=1) as pst:
            sts = pst.tile([16, 2], f32)
            nc.vector.tensor_copy(out=sts[:], in_=statacc[:])
            nc.sync.dma_start(out=stin.ap(), in_=sts[:])
        nc.gpsimd.collective_compute(
            "AllReduce", ALU.add, replica_groups=RG,
            ins=[stin.ap()], outs=[stout.ap()],
        )
        for j in range(8):
            eng = nc.sync if j % 2 == 0 else nc.scalar
            eng.dma_start(out=stripe_s[16 * j:16 * (j + 1), :],
                          in_=h1tall.ap()[16 * j:16 * (j + 1), :])
        if dbg:
            nc.sync.dma_start(out=dbgh1.ap(), in_=h1tall.ap())

        # ---- BN affine from stats (pad nodes contribute relu(b1l) each) ----
        NPAD = NS * NCORES - N
        with tc.tile_pool(name="ph3", bufs=1) as pp3:
            st = pp3.tile([16, 2], f32)
            nc.sync.dma_start(out=st[:], in_=stout.ap())
            rb = pp3.tile([16, 2], f32, tag="rb")
            nc.scalar.activation(out=rb[:, 0:1], in_=rows_s["b1l"][:], func=AF.Relu)
            nc.scalar.square(out=rb[:, 1:2], in_=rb[:, 0:1])
            nc.vector.tensor_scalar(
                out=rb[:], in0=rb[:], scalar1=-float(NPAD), scalar2=None, op0=ALU.mult
            )
            nc.vector.tensor_tensor(out=st[:], in0=st[:], in1=rb[:], op=ALU.add)
            mu = pp3.tile([16, 1], f32, tag="mu")
            nc.vector.tensor_scalar(
                out=mu[:], in0=st[:, 0:1], scalar1=1.0 / N, scalar2=None, op0=ALU.mult
            )
            var = pp3.tile([16, 1], f32, tag="var")
            nc.vector.tensor_scalar(
                out=var[:], in0=st[:, 1:2], scalar1=1.0 / N, scalar2=None, op0=ALU.mult
            )
            musq = pp3.tile([16, 1], f32, tag="musq")
            nc.vector.tensor_tensor(out=musq[:], in0=mu[:], in1=mu[:], op=ALU.mult)
            nc.vector.tensor_tensor(out=var[:], in0=var[:], in1=musq[:], op=ALU.subtract)
            nc.vector.tensor_scalar(
                out=var[:], in0=var[:], scalar1=EPS, scalar2=None, op0=ALU.add
            )
            sd = pp3.tile([16, 1], f32, tag="sd")
            nc.scalar.sqrt(out=sd[:], in_=var[:])
            rstd = pp3.tile([16, 1], f32, tag="rstd")
            nc.vector.reciprocal(out=rstd[:], in_=sd[:])
            nc.vector.tensor_tensor(out=a_col[:], in0=rows_s["g1"][:], in1=rstd[:], op=ALU.mult)
            cc1 = pp3.tile([16, 1], f32, tag="cc1")
            nc.vector.tensor_tensor(out=cc1[:], in0=a_col[:], in1=mu[:], op=ALU.mult)
            nc.vector.tensor_tensor(out=c_col[:], in0=rows_s["be1"][:], in1=cc1[:], op=ALU.subtract)

        # ================= L2 main =================
        ro_pool = top.enter_context(tc.tile_pool(name="rops", bufs=1, space="PSUM"))
        ro_ps = [
            ro_pool.tile([min(P, G - gt * P), 17], f32, tag=f"ro{gt}", name=f"ro{gt}")
            for gt in range(GT)
        ]

        def l2_start(b, psd):
            return False

        def l2_epilogue(b, psd, epp, epps):
            m2 = epp.tile([16, P], bf16, tag="m2", name="m2")
            nc.vector.tensor_scalar(
                out=m2[:], in0=psd[:], scalar1=a_col[:], scalar2=c_col[:],
                op0=ALU.mult, op1=ALU.add,
            )
            bnh1 = epp.tile([16, P], bf16, tag="bnh1", name="bnh1")
            nc.vector.tensor_scalar(
                out=bnh1[:], in0=ytown[:, b * P:(b + 1) * P],
                scalar1=a_col[:], scalar2=c_col[:], op0=ALU.mult, op1=ALU.add,
            )
            h2p = epps.tile([16, P], f32, tag="h2p", name="h2p")
            nc.tensor.matmul(out=h2p[:], lhsT=w2l_b[:], rhs=m2[:], start=True, stop=False,
                             skip_group_check=True)
            nc.tensor.matmul(out=h2p[:], lhsT=w2r_b[:], rhs=bnh1[:], start=False, stop=True,
                             skip_group_check=True)
            h2t = epp.tile([16, P], f32, tag="h2t", name="h2t")
            nc.scalar.activation(out=h2t[:], in_=h2p[:], func=AF.Relu,
                                 bias=rows_s["b2l"][:], scale=1.0)
            h2ep = epps.tile([P, 16], f32, tag="h2ep", name="h2ep")
            nc.tensor.transpose(out=h2ep[:], in_=h2t[:], identity=ident_s[:16, :16])
            h2e = epp.tile([P, 17], f32, tag="h2e", name="h2e")
            nc.vector.tensor_copy(out=h2e[:, 0:16], in_=h2ep[:])
            nc.vector.memset(h2e[:, 16:17], 1.0)
            MTg = epp.tile([P, G], f32, tag="MTg", name="MTg")
            nc.vector.tensor_scalar(
                out=MTg[:], in0=iotag_s[:], scalar1=grel_s[:, b:b + 1],
                scalar2=None, op0=ALU.is_equal,
            )
            for gt in range(GT):
                gsz = min(P, G - gt * P)
                nc.tensor.matmul(
                    out=ro_ps[gt][:], lhsT=MTg[:, gt * P:gt * P + gsz],
                    rhs=h2e[:], start=(b == 0), stop=(b == NB - 1),
                    skip_group_check=True,
                )

        emit_layer(2, l2_epilogue, l2_start)

        # ================= readout =================
        with tc.tile_pool(name="ph5", bufs=1) as pp5, tc.tile_pool(
            name="ph5ps", bufs=1, space="PSUM"
        ) as ps5:
            for gt in range(GT):
                gsz = min(P, G - gt * P)
                ro_s = pp5.tile([P, 17], f32, tag=f"ros{gt}", name=f"ros{gt}")
                nc.vector.tensor_copy(out=ro_s[:gsz, :], in_=ro_ps[gt][:])
                nc.sync.dma_start(out=xein.ap()[gt * P:gt * P + gsz, :], in_=ro_s[:gsz, :])
            nc.gpsimd.collective_compute(
                "AllReduce", ALU.add, replica_groups=RG,
                ins=[xein.ap()], outs=[xeout.ap()],
            )
            if dbg:
                nc.sync.dma_start(out=dbgxe.ap(), in_=xeout.ap())
            xeT = pp5.tile([16, G], f32, tag="xeT")
            for gt in range(GT):
                gsz = min(P, G - gt * P)
                xa = pp5.tile([P, 17], f32, tag=f"xa{gt}", name=f"xa{gt}")
                nc.sync.dma_start(out=xa[:gsz, :], in_=xeout.ap()[gt * P:gt * P + gsz, :])
                cm2 = pp5.tile([P, 1], f32, tag=f"cm2{gt}", name=f"cm2{gt}")
                nc.vector.tensor_scalar_max(out=cm2[:gsz], in0=xa[:gsz, 16:17], scalar1=1.0)
                inv2 = pp5.tile([P, 1], f32, tag=f"inv2{gt}", name=f"inv2{gt}")
                nc.vector.reciprocal(out=inv2[:gsz], in_=cm2[:gsz])
                xe = pp5.tile([P, 16], f32, tag=f"xe{gt}", name=f"xe{gt}")
                nc.vector.tensor_scalar(
                    out=xe[:gsz], in0=xa[:gsz, 0:16], scalar1=inv2[:gsz],
                    scalar2=None, op0=ALU.mult,
                )
                tp = ps5.tile([16, P], f32, tag=f"tp{gt}", name=f"tp{gt}")
                nc.tensor.transpose(out=tp[:, :gsz], in_=xe[:gsz, :], identity=ident_s[:gsz, :gsz])
                nc.vector.tensor_copy(out=xeT[:, gt * P:gt * P + gsz], in_=tp[:, :gsz])

            def bn_t(src_ap, Fd, gl, bl, dest):
                s = pp5.tile([Fd, 1], f32, tag=f"bns{Fd}", name=f"bns{Fd}")
                nc.vector.tensor_reduce(out=s[:], in_=src_ap, axis=mybir.AxisListType.X, op=ALU.add)
                mu5 = pp5.tile([Fd, 1], f32, tag=f"bnmu{Fd}", name=f"bnmu{Fd}")
                nc.vector.tensor_scalar(
                    out=mu5[:], in0=s[:], scalar1=1.0 / G, scalar2=None, op0=ALU.mult
                )
                d = pp5.tile([Fd, G], f32, tag=f"bnd{Fd}", name=f"bnd{Fd}")
                nc.vector.tensor_scalar(
                    out=d[:], in0=src_ap, scalar1=mu5[:], scalar2=None, op0=ALU.subtract
                )
                sq5 = pp5.tile([Fd, G], f32, tag=f"bnsq{Fd}", name=f"bnsq{Fd}")
                nc.vector.tensor_tensor(out=sq5[:], in0=d[:], in1=d[:], op=ALU.mult)
                v = pp5.tile([Fd, 1], f32, tag=f"bnv{Fd}", name=f"bnv{Fd}")
                nc.vector.tensor_reduce(out=v[:], in_=sq5[:], axis=mybir.AxisListType.X, op=ALU.add)
                nc.vector.tensor_scalar(
                    out=v[:], in0=v[:], scalar1=1.0 / G, scalar2=EPS, op0=ALU.mult, op1=ALU.add
                )
                sd5 = pp5.tile([Fd, 1], f32, tag=f"bnsd{Fd}", name=f"bnsd{Fd}")
                nc.scalar.sqrt(out=sd5[:], in_=v[:])
                rs5 = pp5.tile([Fd, 1], f32, tag=f"bnrs{Fd}", name=f"bnrs{Fd}")
                nc.vector.reciprocal(out=rs5[:], in_=sd5[:])
                sc5 = pp5.tile([Fd, 1], f32, tag=f"bnsc{Fd}", name=f"bnsc{Fd}")
                nc.vector.tensor_tensor(out=sc5[:], in0=gl, in1=rs5[:], op=ALU.mult)
                nc.vector.tensor_scalar(
                    out=dest, in0=d[:], scalar1=sc5[:], scalar2=bl, op0=ALU.mult, op1=ALU.add
                )

            bn1 = pp5.tile([16, G], f32, tag="bn1")
            bn_t(xeT[:], 16, cols_s["gl1"][:], cols_s["bl1"][:], bn1[:])
            z1p = ps5.tile([16, G], f32, tag="z1p")
            nc.tensor.matmul(out=z1p[:], lhsT=Ws_s["W1"][:], rhs=bn1[:], start=True, stop=True)
            zs1 = pp5.tile([16, G], f32, tag="zs1")
            nc.scalar.activation(out=zs1[:], in_=z1p[:], func=AF.Relu, bias=cols_s["bW1"][:], scale=1.0)
            bn2a = pp5.tile([16, G], f32, tag="bn2a")
            bn_t(zs1[:], 16, cols_s["gl2a"][:], cols_s["bl2a"][:], bn2a[:])
            bn2b = pp5.tile([16, G], f32, tag="bn2b")
            bn_t(xeT[:], 16, cols_s["gl2b"][:], cols_s["bl2b"][:], bn2b[:])
            z2p = ps5.tile([16, G], f32, tag="z2p")
            nc.tensor.matmul(out=z2p[:], lhsT=Ws_s["W2a"][:], rhs=bn2a[:], start=True, stop=False)
            nc.tensor.matmul(out=z2p[:], lhsT=Ws_s["W2b"][:], rhs=bn2b[:], start=False, stop=True)
            zs2 = pp5.tile([16, G], f32, tag="zs2")
            nc.scalar.activation(out=zs2[:], in_=z2p[:], func=AF.Relu, bias=cols_s["bW2"][:], scale=1.0)
            bn3a = pp5.tile([16, G], f32, tag="bn3a")
            bn_t(zs2[:], 16, cols_s["gl3a"][:], cols_s["bl3a"][:], bn3a[:])
            bn3b = pp5.tile([16, G], f32, tag="bn3b")
            bn_t(xeT[:], 16, cols_s["gl3b"][:], cols_s["bl3b"][:], bn3b[:])
            z3p = ps5.tile([16, G], f32, tag="z3p")
            nc.tensor.matmul(out=z3p[:], lhsT=Ws_s["W3a"][:], rhs=bn3a[:], start=True, stop=False)
            nc.tensor.matmul(out=z3p[:], lhsT=Ws_s["W3b"][:], rhs=bn3b[:], start=False, stop=True)
            z3 = pp5.tile([16, G], f32, tag="z3")
            nc.scalar.activation(out=z3[:], in_=z3p[:], func=AF.Relu, bias=cols_s["bW3"][:], scale=1.0)
            ofp = ps5.tile([1, G], f32, tag="ofp")
            nc.tensor.matmul(out=ofp[:], lhsT=Ws_s["Wf"][:], rhs=z3[:], start=True, stop=True)
            ofs = pp5.tile([1, G], f32, tag="ofs")
            nc.vector.tensor_scalar(
                out=ofs[:], in0=ofp[:], scalar1=cols_s["bWf"][:], scalar2=None, op0=ALU.add
            )
            nc.sync.dma_start(out=out_t.ap(), in_=ofs[:])

    nc.compile()
    return nc


def run(inputs, n_cores=8, G=256, cfg_overrides=None, trace=False, enable_asserts=False):
    from concourse.bass_utils import run_bass_kernel_spmd

    in_maps, cfg = build_host_data(
        inputs["x"], inputs["edge_index"], inputs["batch"], n_cores, G
    )
    if cfg_overrides:
        cfg.update(cfg_overrides)
    add_weights(in_maps, inputs)
    nc = build_program(cfg, enable_asserts=enable_asserts)
    res = run_bass_kernel_spmd(nc, in_maps, core_ids=list(range(n_cores)), trace=trace)
    out = res.results[0]["out"].reshape(G, 1)
    return out, res, cfg


def kernel(**inputs):
    """Full inputs -> full [256, 1] output. Shards internally across 8 cores."""
    out, _, _ = run(inputs, n_cores=8, G=256)
    return np.asarray(out, np.float32)


# revision 17
# speedup vs baseline: 1.2485x; 1.2485x over previous
"""Trainium2 Bass kernel for nn_D2RLCritic (gnn_message_passing).

Self-contained: kernel(**inputs) takes the FULL unsharded inputs (as from
setup_inputs()) and returns the FULL [256, 1] output, running an SPMD Bass
kernel across 8 NeuronCores.

Design: dst-sharded graph (12544 nodes/core, 98 blocks of 128). Per-edge
feature fetches use gpsimd ap_gather from SBUF-resident transposed feature
stripes ([128, 12544]: partition 16r+f = feature f of node range r), with
per-16-partition-group index streams. L1 projects x@w1l per node first, so
both layers gather 16-dim rows. Gathered columns are transposed on PE into
slot-major E tiles; a bf16 one-hot (dst within block) matmul accumulates the
segment sum in PSUM. Degrees/masks are host-precomputed index tables.
"""

import numpy as np
from contextlib import ExitStack

from concourse import bass, bacc, mybir, tile
from concourse.mybir import AluOpType as ALU
from concourse.mybir import ActivationFunctionType as AF

P = 128
NR = 8
dt = mybir.dt
EPS = 1e-5
CW = 4096


def build_host_data(x, edge_index, batch, n_cores, G):
    assert n_cores == NR
    x = np.ascontiguousarray(np.asarray(x, np.float32))
    src_g = np.asarray(edge_index[0], np.int64)
    dst_g = np.asarray(edge_index[1], np.int64)
    batch = np.asarray(batch, np.int64)
    N0, F = x.shape
    NS = ((N0 + NR * P - 1) // (NR * P)) * P  # 12544
    Npad = NS * NR
    NB = NS // P

    xp = np.zeros((Npad, F), np.float32)
    xp[:N0] = x
    deg = np.bincount(dst_g, minlength=Npad).astype(np.int64)
    batchp = np.full(Npad, -1, np.int64)
    batchp[:N0] = batch

    # per-core edge sort and cell counts
    per_s, per_d, per_cnt = [], [], []
    for k in range(NR):
        m = (dst_g >= k * NS) & (dst_g < (k + 1) * NS)
        s = src_g[m]
        d = dst_g[m] - k * NS
        blk = d >> 7
        rng = s // NS
        order = np.lexsort((s, rng, blk))
        s, d, blk, rng = s[order], d[order], blk[order], rng[order]
        cnt = np.zeros((NB, NR), np.int64)
        np.add.at(cnt, (blk, rng), 1)
        per_s.append(s)
        per_d.append(d)
        per_cnt.append(cnt)
    cnts = np.stack(per_cnt)            # [NR_cores, NB, NR]
    W = cnts.max(axis=0)                # [NB, NR]

    S = np.zeros((NB, NR), np.int64)    # stream offset of cell (b, r)
    S[1:] = np.cumsum(W, axis=0)[:-1]
    L_r = S[-1] + W[-1]
    L = int(((L_r.max() + P - 1) // P) * P)
    NCH = (L + CW - 1) // CW

    # pieces: per block, ordered list of (r, window, drel_col); windows are
    # 128-col spans of all 8 streams (one transposed square serves 8 ranges)
    pieces = []
    npiece = 0
    for b in range(NB):
        plist = []
        for r in range(NR):
            a, z = int(S[b, r]), int(S[b, r] + W[b, r])
            for win in range(a // P, (z + P - 1) // P):
                plist.append((r, win, npiece))
                npiece += 1
        plist.sort(key=lambda t: (t[1], t[0]))
        pieces.append(plist)
    NPIECE = npiece
    in_maps = []
    for k in range(NR):
        s, d, cnt = per_s[k], per_d[k], per_cnt[k]
        # cell start offsets in the sorted edge array
        estart = np.concatenate([[0], np.cumsum(cnt.ravel())])[:-1].reshape(NB, NR)
        # gather index streams, wrapped per 16-partition group
        apg = np.zeros((P, L // 16), np.int16)
        dstrel = np.full((NR, L), -1, np.int64)
        for r in range(NR):
            stream = np.zeros(L, np.int64)
            for b in range(NB):
                n_e = int(cnt[b, r])
                if n_e:
                    e0 = estart[b, r]
                    stream[S[b, r]: S[b, r] + n_e] = s[e0: e0 + n_e] - r * NS
                    dstrel[r, S[b, r]: S[b, r] + n_e] = d[e0: e0 + n_e] - b * P
            apg[16 * r: 16 * (r + 1), :] = (
                stream.reshape(L // 16, 16).T.astype(np.int16)
            )
        # mtbig: [128, NPIECE*128] bf16 inverse-degree-scaled one-hots:
        # mt[p, pc*128+dd] = 1/deg(dst) if slot p of piece pc maps to block
        # offset dd, else 0
        import ml_dtypes
        drel = np.full((P, NPIECE), -1, np.int64)
        pblk = np.zeros(NPIECE, np.int64)
        for b in range(NB):
            for (r, win, pc) in pieces[b]:
                a, z = int(S[b, r]), int(S[b, r] + W[b, r])
                lo, hi = win * P, (win + 1) * P
                aa, zz = max(a, lo), min(z, hi)
                col = np.full(P, -1, np.int64)
                col[aa - lo: zz - lo] = dstrel[r, aa: zz]
                drel[:, pc] = col
                pblk[pc] = b
        onehot = (drel[:, :, None] == np.arange(P)[None, None, :])
        nodes_all = np.arange(NS) + k * NS
        invd_full = (1.0 / np.maximum(deg[nodes_all], 1)).astype(np.float32)
        inv_pc = invd_full.reshape(NB, P)[pblk]          # [NPIECE, 128]
        mtbig = (onehot * inv_pc[None, :, :]).astype(ml_dtypes.bfloat16)
        mtbig = mtbig.reshape(P, NPIECE * P)
        nodes = np.arange(NS) + k * NS
        grel = np.where(nodes < N0, batchp[nodes], -1).astype(np.float32)
        in_maps.append(dict(
            xown=xp[k * NS:(k + 1) * NS],
            apgidx=apg,
            mtbig=mtbig,
            grel=grel.reshape(NB, P).T.copy(),
        ))

    cfg = dict(N=N0, NS=NS, NB=NB, F=F, G=G, NPIECE=NPIECE, L=L, NCH=NCH,
               pieces=pieces, n_cores=NR)
    return in_maps, cfg


def add_weights(in_maps, inputs):
    f32 = np.float32
    w = {}
    w["w1cat"] = np.concatenate(
        [np.asarray(inputs["w1l"], f32), np.asarray(inputs["w1r"], f32)], axis=1
    )  # [64, 32]
    w["w2l"] = np.asarray(inputs["w2l"], f32)
    w["w2r"] = np.asarray(inputs["w2r"], f32)
    for name in ("b1l", "b2l", "g1", "be1"):
        w[name] = np.asarray(inputs[name], f32).reshape(16, 1)
    for name in ("gl1", "bl1", "bW1", "bW2", "bW3"):
        w[name] = np.asarray(inputs[name], f32).reshape(16, 1)
    w["bWf"] = np.asarray(inputs["bWf"], f32).reshape(1, 1)
    for name in ("gl2", "bl2", "gl3", "bl3"):
        v = np.asarray(inputs[name], f32).reshape(32, 1)
        w[name + "a"], w[name + "b"] = v[:16].copy(), v[16:].copy()
    w["W1"] = np.asarray(inputs["W1"], f32)
    w["Wf"] = np.asarray(inputs["Wf"], f32)
    for name in ("W2", "W3"):
        v = np.asarray(inputs[name], f32)
        w[name + "a"], w[name + "b"] = v[:16].copy(), v[16:].copy()
    for m in in_maps:
        m.update(w)
    return in_maps


def build_program(cfg, enable_asserts=False):
    NCORES = cfg["n_cores"]
    N, NS, NB, F, G = cfg["N"], cfg["NS"], cfg["NB"], cfg["F"], cfg["G"]
    NPIECE, L, NCH = cfg["NPIECE"], cfg["L"], cfg["NCH"]
    pieces = cfg["pieces"]
    GT = (G + P - 1) // P
    f32, bf16 = dt.float32, dt.bfloat16

    nc = bacc.Bacc(
        "TRN2", target_bir_lowering=False, debug=False,
        enable_asserts=enable_asserts, num_devices=NCORES,
    )
    RG = [list(range(NCORES))]

    xown_in = nc.dram_tensor("xown", [NS, F], f32, kind="ExternalInput")
    apg_in = nc.dram_tensor("apgidx", [P, L // 16], dt.int16, kind="ExternalInput")
    mtbig_in = nc.dram_tensor("mtbig", [P, NPIECE * P], dt.bfloat16, kind="ExternalInput")
    grel_in = nc.dram_tensor("grel", [P, NB], f32, kind="ExternalInput")
    w1cat_in = nc.dram_tensor("w1cat", [F, 32], f32, kind="ExternalInput")
    w2l_in = nc.dram_tensor("w2l", [16, 16], f32, kind="ExternalInput")
    w2r_in = nc.dram_tensor("w2r", [16, 16], f32, kind="ExternalInput")
    row_ins = {
        name: nc.dram_tensor(name, [16, 1], f32, kind="ExternalInput")
        for name in ("b1l", "b2l", "g1", "be1")
    }
    col_names = ("gl1", "bl1", "bW1", "gl2a", "gl2b", "bl2a", "bl2b",
                 "gl3a", "gl3b", "bl3a", "bl3b", "bW2", "bW3")
    col_ins = {
        name: nc.dram_tensor(name, [16, 1], f32, kind="ExternalInput")
        for name in col_names
    }
    col_ins["bWf"] = nc.dram_tensor("bWf", [1, 1], f32, kind="ExternalInput")
    W_ins = {
        name: nc.dram_tensor(name, [16, shp1], f32, kind="ExternalInput")
        for name, shp1 in (
            ("W1", 16), ("W2a", 16), ("W2b", 16), ("W3a", 16), ("W3b", 16), ("Wf", 1),
        )
    }
    out_t = nc.dram_tensor("out", [1, G], f32, kind="ExternalOutput")
    dbg = cfg.get("debug")
    if dbg:
        dbgy = nc.dram_tensor("dbgy", [P, NS], f32, kind="ExternalOutput")
        dbgh1 = nc.dram_tensor("dbgh1", [P, NS], f32, kind="ExternalOutput")
        dbgxe = nc.dram_tensor("dbgxe", [G, 17], f32, kind="ExternalOutput")


    y1tsh = nc.dram_tensor("y1tsh", [16, NS], f32, kind="Internal")
    y1tall = nc.dram_tensor("y1tall", [P, NS], f32, kind="Internal", addr_space="Shared")
    h1tsh = nc.dram_tensor("h1tsh", [16, NS], f32, kind="Internal")
    h1tall = nc.dram_tensor("h1tall", [P, NS], f32, kind="Internal", addr_space="Shared")
    stin = nc.dram_tensor("stin", [16, 2], f32, kind="Internal")
    stout = nc.dram_tensor("stout", [16, 2], f32, kind="Internal", addr_space="Shared")
    xein = nc.dram_tensor("xein", [G, 17], f32, kind="Internal")
    xeout = nc.dram_tensor("xeout", [G, 17], f32, kind="Internal", addr_space="Shared")

    iota128_t = nc.inline_tensor(
        np.broadcast_to(np.arange(P, dtype=np.float32), (P, P)).copy(), "iota128"
    )
    iotag_t = nc.inline_tensor(
        np.broadcast_to(np.arange(G, dtype=np.float32), (P, G)).copy(), "iotag"
    )
    ident_t = nc.inline_tensor(np.eye(P, dtype=np.float32), "ident")

    with tile.TileContext(nc) as tc, ExitStack() as top:
        persist = top.enter_context(tc.tile_pool(name="persist", bufs=1))

        iota_f = persist.tile([P, P], f32)
        nc.sync.dma_start(out=iota_f[:], in_=iota128_t.ap())
        iotag_s = persist.tile([P, G], f32)
        nc.sync.dma_start(out=iotag_s[:], in_=iotag_t.ap())
        ident_s = persist.tile([P, P], f32)
        nc.sync.dma_start(out=ident_s[:], in_=ident_t.ap())
        apg_s = persist.tile([P, L // 16], dt.int16)
        nc.sync.dma_start(out=apg_s[:], in_=apg_in.ap())
        grel_s = persist.tile([P, NB], f32)
        nc.sync.dma_start(out=grel_s[:], in_=grel_in.ap())
        w1cat_s = persist.tile([F, 32], f32)
        nc.sync.dma_start(out=w1cat_s[:], in_=w1cat_in.ap())
        w2l_s = persist.tile([16, 16], f32)
        nc.sync.dma_start(out=w2l_s[:], in_=w2l_in.ap())
        w2r_s = persist.tile([16, 16], f32)
        nc.sync.dma_start(out=w2r_s[:], in_=w2r_in.ap())
        rows_s = {}
        for name, t in row_ins.items():
            rows_s[name] = persist.tile([16, 1], f32, tag=f"row_{name}", name=f"row_{name}")
            nc.sync.dma_start(out=rows_s[name][:], in_=t.ap())
        cols_s = {}
        for name, t in col_ins.items():
            cols_s[name] = persist.tile(list(t.shape), f32, tag=f"col_{name}", name=f"col_{name}")
            nc.sync.dma_start(out=cols_s[name][:], in_=t.ap())
        Ws_s = {}
        for name, t in W_ins.items():
            Ws_s[name] = persist.tile(list(t.shape), f32, tag=f"W_{name}", name=f"W_{name}")
            nc.sync.dma_start(out=Ws_s[name][:], in_=t.ap())

        w1cat_b = persist.tile([F, 32], bf16)
        nc.vector.tensor_copy(out=w1cat_b[:], in_=w1cat_s[:])
        w2l_b = persist.tile([16, 16], bf16)
        nc.vector.tensor_copy(out=w2l_b[:], in_=w2l_s[:])
        w2r_b = persist.tile([16, 16], bf16)
        nc.vector.tensor_copy(out=w2r_b[:], in_=w2r_s[:])
        ident16_b = persist.tile([16, 16], bf16)
        nc.vector.tensor_copy(out=ident16_b[:], in_=ident_s[:16, :16])

        stripe_s = persist.tile([P, NS], f32)        # y1T then h1T (gathered)
        ytown = persist.tile([16, NS], f32)          # y1T own, then h1T own
        xrbt_own = persist.tile([16, NS], bf16)      # (x @ w1r + b1l)^T own
        statacc = persist.tile([16, 2], f32)
        nc.vector.memset(statacc[:], 0.0)
        a_col = persist.tile([16, 1], f32, tag="a_col")
        c_col = persist.tile([16, 1], f32, tag="c_col")

        # warmup: load ap_gather ucode early
        with tc.tile_pool(name="warm", bufs=1) as wp:
            wi = wp.tile([P, 16], dt.int16)
            nc.gpsimd.memset(wi[:], 0)
            wo = wp.tile([P, 16], f32)
            nc.gpsimd.ap_gather(
                out_ap=wo[:], in_ap=iota_f[:], idxs_ap=wi[:, 0:1],
                channels=P, num_elems=P, d=1, num_idxs=16,
            )

        # ================= L1 prep: y1T own + xrbT =================
        with tc.tile_pool(name="p1", bufs=3) as pl, tc.tile_pool(
            name="p1ps", bufs=2, space="PSUM"
        ) as ps1, tc.tile_pool(name="p1s", bufs=3) as sb1:
            for b in range(NB):
                xb = pl.tile([P, F], f32, tag="xb")
                nc.sync.dma_start(out=xb[:], in_=xown_in.ap()[b * P:(b + 1) * P, :])
                xTp = ps1.tile([F, P], f32, tag="xTp", name="xTp")
                nc.tensor.transpose(out=xTp[:], in_=xb[:], identity=ident_s[:])
                xT_s = sb1.tile([F, P], bf16, tag="xTs")
                nc.vector.tensor_copy(out=xT_s[:], in_=xTp[:])
                y1p = ps1.tile([16, P], f32, tag="y1p", name="y1p")
                nc.tensor.matmul(out=y1p[:], lhsT=w1cat_b[:, 0:16], rhs=xT_s[:],
                                 start=True, stop=True)
                nc.vector.tensor_copy(out=ytown[:, b * P:(b + 1) * P], in_=y1p[:])
                xrp = ps1.tile([16, P], f32, tag="xrp", name="xrp")
                nc.tensor.matmul(out=xrp[:], lhsT=w1cat_b[:, 16:32], rhs=xT_s[:],
                                 start=True, stop=True)
                nc.vector.tensor_scalar(
                    out=xrbt_own[:, b * P:(b + 1) * P], in0=xrp[:],
                    scalar1=rows_s["b1l"][:], scalar2=None, op0=ALU.add,
                )
        nc.sync.dma_start(out=y1tsh.ap(), in_=ytown[:])
        nc.gpsimd.collective_compute(
            "AllGather", ALU.bypass, replica_groups=RG,
            ins=[y1tsh.ap()], outs=[y1tall.ap()],
        )
        nc.sync.dma_start(out=stripe_s[:], in_=y1tall.ap())
        if dbg:
            nc.sync.dma_start(out=dbgy.ap(), in_=y1tall.ap())

        # ================= shared edge-layer emitter =================
        WPC = CW // P   # windows per chunk
        MTB = 16        # one-hot pieces per DMA batch
        fence_pool = top.enter_context(tc.tile_pool(name="fence", bufs=2))

        def pool_fence():
            """Order later gpsimd work after the stripe/idx loads: gpsimd is
            in-order, and this op's reads are dependency-tracked."""
            fp = fence_pool.tile([1, 4], f32, tag="fence", name="fence")
            nc.gpsimd.tensor_tensor(out=fp[:], in0=stripe_s[0:1, 0:4],
                                    in1=apg_s[0:1, 0:8].bitcast(f32),
                                    op=ALU.add)

        def emit_layer(layer, epilogue, start_extra):
            """Gather + window transposes + psdT accumulation per block.

            psdT[f, d] = sum over pieces of E_piece^T @ MTinv_piece; the
            host-prepared MTinv folds 1/deg. start_extra(b, psd) may emit an
            initial accumulating matmul (returns True if it started the
            group)."""
            pool_fence()
            with tc.tile_pool(name=f"ch{layer}", bufs=3) as chp, tc.tile_pool(
                name=f"sqps{layer}", bufs=2, space="PSUM"
            ) as sqps, tc.tile_pool(name=f"sq{layer}", bufs=8) as sqp, tc.tile_pool(
                name=f"mt{layer}", bufs=3
            ) as mtp, tc.tile_pool(name=f"psd{layer}", bufs=2, space="PSUM") as psdp, \
                 tc.tile_pool(name=f"ep{layer}", bufs=3) as epp, tc.tile_pool(
                name=f"epps{layer}", bufs=1, space="PSUM"
            ) as epps:
                chunks = {}
                squares = {}
                mtts = {}
                next_ch = 0

                def ensure_window(win):
                    nonlocal next_ch
                    if win in squares:
                        return
                    while next_ch <= win // WPC and next_ch < NCH:
                        cw = min(CW, L - next_ch * CW)
                        ct = chp.tile([P, CW], f32, tag="chunk")
                        nc.gpsimd.ap_gather(
                            out_ap=ct[:, 0:cw], in_ap=stripe_s[:],
                            idxs_ap=apg_s[:, next_ch * (CW // 16):
                                          next_ch * (CW // 16) + cw // 16],
                            channels=P, num_elems=NS, d=1, num_idxs=cw,
                        )
                        chunks[next_ch] = ct
                        next_ch += 1
                    cc = (win % WPC) * P
                    sq_ps = sqps.tile([P, P], f32, tag="sqps", name="sqps")
                    nc.tensor.transpose(
                        out=sq_ps[:], in_=chunks[win // WPC][:, cc:cc + P],
                        identity=ident_s[:],
                    )
                    sq = sqp.tile([P, P], bf16, tag="sq")
                    nc.scalar.activation(out=sq[:], in_=sq_ps[:], func=AF.Copy)
                    squares[win] = sq

                def ensure_mt(pc):
                    g = pc // MTB
                    if g not in mtts:
                        g0 = g * MTB * P
                        gw = min(MTB * P, NPIECE * P - g0)
                        mt = mtp.tile([P, MTB * P], bf16, tag="mtb")
                        nc.sync.dma_start(out=mt[:, 0:gw],
                                          in_=mtbig_in.ap()[:, g0:g0 + gw])
                        mtts[g] = mt
                    return mtts[g], (pc % MTB) * P

                for b in range(NB):
                    for (r, win, pc) in pieces[b]:
                        ensure_window(win)
                        ensure_mt(pc)
                    psd = psdp.tile([16, P], f32, tag="psd", name="psd")
                    started = start_extra(b, psd)
                    np_b = len(pieces[b])
                    for i, (r, win, pc) in enumerate(pieces[b]):
                        mt, mo = ensure_mt(pc)
                        nc.tensor.matmul(
                            out=psd[:], lhsT=squares[win][:, 16 * r: 16 * (r + 1)],
                            rhs=mt[:, mo: mo + P],
                            start=(i == 0 and not started), stop=(i == np_b - 1),
                            skip_group_check=True,
                        )
                    epilogue(b, psd, epp, epps)

        # ================= L1 main =================
        def l1_start(b, psd):
            nc.tensor.matmul(
                out=psd[:], lhsT=ident16_b[:],
                rhs=xrbt_own[:, b * P:(b + 1) * P],
                start=True, stop=False, skip_group_check=True,
            )
            return True

        def l1_epilogue(b, psd, epp, epps):
            h1t = epp.tile([16, P], f32, tag="h1t", name="h1t")
            nc.scalar.activation(out=h1t[:], in_=psd[:], func=AF.Relu)
            nc.vector.tensor_copy(out=ytown[:, b * P:(b + 1) * P], in_=h1t[:])
            sq1 = epp.tile([16, P], f32, tag="sq1", name="sq1")
            nc.scalar.square(out=sq1[:], in_=h1t[:])
            red = epp.tile([16, 2], f32, tag="red", name="red")
            nc.vector.tensor_reduce(out=red[:, 0:1], in_=h1t[:],
                                    axis=mybir.AxisListType.X, op=ALU.add)
            nc.vector.tensor_reduce(out=red[:, 1:2], in_=sq1[:],
                                    axis=mybir.AxisListType.X, op=ALU.add)
            nc.vector.tensor_tensor(out=statacc[:], in0=statacc[:], in1=red[:],
                                    op=ALU.add)

        emit_layer(1, l1_epilogue, l1_start)

        nc.sync.dma_start(out=h1tsh.ap(), in_=ytown[:])
        nc.gpsimd.collective_compute(
            "AllGather", ALU.bypass, replica_groups=RG,
            ins=[h1tsh.ap()], outs=[h1tall.ap()],
        )
        with tc.tile_pool(name="st", bufs=1) as pst:
            sts = pst.tile([16, 2], f32)
            nc.vector.tensor_copy(out=sts[:], in_=statacc[:])
            nc.sync.dma_start(out=stin.ap(), in_=sts[:])
        nc.gpsimd.collective_compute(
            "AllReduce", ALU.add, replica_groups=RG,
            ins=[stin.ap()], outs=[stout.ap()],
        )
        nc.sync.dma_start(out=stripe_s[:], in_=h1tall.ap())
        if dbg:
            nc.sync.dma_start(out=dbgh1.ap(), in_=h1tall.ap())

        # ---- BN affine from stats (pad nodes contribute relu(b1l) each) ----
        NPAD = NS * NCORES - N
        with tc.tile_pool(name="ph3", bufs=1) as pp3:
            st = pp3.tile([16, 2], f32)
            nc.sync.dma_start(out=st[:], in_=stout.ap())
            rb = pp3.tile([16, 2], f32, tag="rb")
            nc.scalar.activation(out=rb[:, 0:1], in_=rows_s["b1l"][:], func=AF.Relu)
            nc.scalar.square(out=rb[:, 1:2], in_=rb[:, 0:1])
            nc.vector.tensor_scalar(
                out=rb[:], in0=rb[:], scalar1=-float(NPAD), scalar2=None, op0=ALU.mult
            )
            nc.vector.tensor_tensor(out=st[:], in0=st[:], in1=rb[:], op=ALU.add)
            mu = pp3.tile([16, 1], f32, tag="mu")
            nc.vector.tensor_scalar(
                out=mu[:], in0=st[:, 0:1], scalar1=1.0 / N, scalar2=None, op0=ALU.mult
            )
            var = pp3.tile([16, 1], f32, tag="var")
            nc.vector.tensor_scalar(
                out=var[:], in0=st[:, 1:2], scalar1=1.0 / N, scalar2=None, op0=ALU.mult
            )
            musq = pp3.tile([16, 1], f32, tag="musq")
            nc.vector.tensor_tensor(out=musq[:], in0=mu[:], in1=mu[:], op=ALU.mult)
            nc.vector.tensor_tensor(out=var[:], in0=var[:], in1=musq[:], op=ALU.subtract)
            nc.vector.tensor_scalar(
                out=var[:], in0=var[:], scalar1=EPS, scalar2=None, op0=ALU.add
            )
            sd = pp3.tile([16, 1], f32, tag="sd")
            nc.scalar.sqrt(out=sd[:], in_=var[:])
            rstd = pp3.tile([16, 1], f32, tag="rstd")
            nc.vector.reciprocal(out=rstd[:], in_=sd[:])
            nc.vector.tensor_tensor(out=a_col[:], in0=rows_s["g1"][:], in1=rstd[:], op=ALU.mult)
            cc1 = pp3.tile([16, 1], f32, tag="cc1")
            nc.vector.tensor_tensor(out=cc1[:], in0=a_col[:], in1=mu[:], op=ALU.mult)
            nc.vector.tensor_tensor(out=c_col[:], in0=rows_s["be1"][:], in1=cc1[:], op=ALU.subtract)

        # ================= L2 main =================
        ro_pool = top.enter_context(tc.tile_pool(name="rops", bufs=1, space="PSUM"))
        ro_ps = [
            ro_pool.tile([min(P, G - gt * P), 17], f32, tag=f"ro{gt}", name=f"ro{gt}")
            for gt in range(GT)
        ]

        def l2_start(b, psd):
            return False

        def l2_epilogue(b, psd, epp, epps):
            m2 = epp.tile([16, P], bf16, tag="m2", name="m2")
            nc.vector.tensor_scalar(
                out=m2[:], in0=psd[:], scalar1=a_col[:], scalar2=c_col[:],
                op0=ALU.mult, op1=ALU.add,
            )
            bnh1 = epp.tile([16, P], bf16, tag="bnh1", name="bnh1")
            nc.vector.tensor_scalar(
                out=bnh1[:], in0=ytown[:, b * P:(b + 1) * P],
                scalar1=a_col[:], scalar2=c_col[:], op0=ALU.mult, op1=ALU.add,
            )
            h2p = epps.tile([16, P], f32, tag="h2p", name="h2p")
            nc.tensor.matmul(out=h2p[:], lhsT=w2l_b[:], rhs=m2[:], start=True, stop=False,
                             skip_group_check=True)
            nc.tensor.matmul(out=h2p[:], lhsT=w2r_b[:], rhs=bnh1[:], start=False, stop=True,
                             skip_group_check=True)
            h2t = epp.tile([16, P], f32, tag="h2t", name="h2t")
            nc.scalar.activation(out=h2t[:], in_=h2p[:], func=AF.Relu,
                                 bias=rows_s["b2l"][:], scale=1.0)
            h2ep = epps.tile([P, 16], f32, tag="h2ep", name="h2ep")
            nc.tensor.transpose(out=h2ep[:], in_=h2t[:], identity=ident_s[:16, :16])
            h2e = epp.tile([P, 17], f32, tag="h2e", name="h2e")
            nc.vector.tensor_copy(out=h2e[:, 0:16], in_=h2ep[:])
            nc.vector.memset(h2e[:, 16:17], 1.0)
            MTg = epp.tile([P, G], f32, tag="MTg", name="MTg")
            nc.vector.tensor_scalar(
                out=MTg[:], in0=iotag_s[:], scalar1=grel_s[:, b:b + 1],
                scalar2=None, op0=ALU.is_equal,
            )
            for gt in range(GT):
                gsz = min(P, G - gt * P)
                nc.tensor.matmul(
                    out=ro_ps[gt][:], lhsT=MTg[:, gt * P:gt * P + gsz],
                    rhs=h2e[:], start=(b == 0), stop=(b == NB - 1),
                    skip_group_check=True,
                )

        emit_layer(2, l2_epilogue, l2_start)

        # ================= readout =================
        with tc.tile_pool(name="ph5", bufs=1) as pp5, tc.tile_pool(
            name="ph5ps", bufs=1, space="PSUM"
        ) as ps5:
            for gt in range(GT):
                gsz = min(P, G - gt * P)
                ro_s = pp5.tile([P, 17], f32, tag=f"ros{gt}", name=f"ros{gt}")
                nc.vector.tensor_copy(out=ro_s[:gsz, :], in_=ro_ps[gt][:])
                nc.sync.dma_start(out=xein.ap()[gt * P:gt * P + gsz, :], in_=ro_s[:gsz, :])
            nc.gpsimd.collective_compute(
                "AllReduce", ALU.add, replica_groups=RG,
                ins=[xein.ap()], outs=[xeout.ap()],
            )
            if dbg:
                nc.sync.dma_start(out=dbgxe.ap(), in_=xeout.ap())
            xeT = pp5.tile([16, G], f32, tag="xeT")
            for gt in range(GT):
                gsz = min(P, G - gt * P)
                xa = pp5.tile([P, 17], f32, tag=f"xa{gt}", name=f"xa{gt}")
                nc.sync.dma_start(out=xa[:gsz, :], in_=xeout.ap()[gt * P:gt * P + gsz, :])
                cm2 = pp5.tile([P, 1], f32, tag=f"cm2{gt}", name=f"cm2{gt}")
                nc.vector.tensor_scalar_max(out=cm2[:gsz], in0=xa[:gsz, 16:17], scalar1=1.0)
                inv2 = pp5.tile([P, 1], f32, tag=f"inv2{gt}", name=f"inv2{gt}")
                nc.vector.reciprocal(out=inv2[:gsz], in_=cm2[:gsz])
                xe = pp5.tile([P, 16], f32, tag=f"xe{gt}", name=f"xe{gt}")
                nc.vector.tensor_scalar(
                    out=xe[:gsz], in0=xa[:gsz, 0:16], scalar1=inv2[:gsz],
                    scalar2=None, op0=ALU.mult,
                )
                tp = ps5.tile([16, P], f32, tag=f"tp{gt}", name=f"tp{gt}")
                nc.tensor.transpose(out=tp[:, :gsz], in_=xe[:gsz, :], identity=ident_s[:gsz, :gsz])
                nc.vector.tensor_copy(out=xeT[:, gt * P:gt * P + gsz], in_=tp[:, :gsz])

            def bn_t(src_ap, Fd, gl, bl, dest):
                s = pp5.tile([Fd, 1], f32, tag=f"bns{Fd}", name=f"bns{Fd}")
                nc.vector.tensor_reduce(out=s[:], in_=src_ap, axis=mybir.AxisListType.X, op=ALU.add)
                mu5 = pp5.tile([Fd, 1], f32, tag=f"bnmu{Fd}", name=f"bnmu{Fd}")
                nc.vector.tensor_scalar(
                    out=mu5[:], in0=s[:], scalar1=1.0 / G, scalar2=None, op0=ALU.mult
                )
                d = pp5.tile([Fd, G], f32, tag=f"bnd{Fd}", name=f"bnd{Fd}")
                nc.vector.tensor_scalar(
                    out=d[:], in0=src_ap, scalar1=mu5[:], scalar2=None, op0=ALU.subtract
                )
                sq5 = pp5.tile([Fd, G], f32, tag=f"bnsq{Fd}", name=f"bnsq{Fd}")
                nc.vector.tensor_tensor(out=sq5[:], in0=d[:], in1=d[:], op=ALU.mult)
                v = pp5.tile([Fd, 1], f32, tag=f"bnv{Fd}", name=f"bnv{Fd}")
                nc.vector.tensor_reduce(out=v[:], in_=sq5[:], axis=mybir.AxisListType.X, op=ALU.add)
                nc.vector.tensor_scalar(
                    out=v[:], in0=v[:], scalar1=1.0 / G, scalar2=EPS, op0=ALU.mult, op1=ALU.add
                )
                sd5 = pp5.tile([Fd, 1], f32, tag=f"bnsd{Fd}", name=f"bnsd{Fd}")
                nc.scalar.sqrt(out=sd5[:], in_=v[:])
                rs5 = pp5.tile([Fd, 1], f32, tag=f"bnrs{Fd}", name=f"bnrs{Fd}")
                nc.vector.reciprocal(out=rs5[:], in_=sd5[:])
                sc5 = pp5.tile([Fd, 1], f32, tag=f"bnsc{Fd}", name=f"bnsc{Fd}")
                nc.vector.tensor_tensor(out=sc5[:], in0=gl, in1=rs5[:], op=ALU.mult)
                nc.vector.tensor_scalar(
                    out=dest, in0=d[:], scalar1=sc5[:], scalar2=bl, op0=ALU.mult, op1=ALU.add
                )

            bn1 = pp5.tile([16, G], f32, tag="bn1")
            bn_t(xeT[:], 16, cols_s["gl1"][:], cols_s["bl1"][:], bn1[:])
            z1p = ps5.tile([16, G], f32, tag="z1p")
            nc.tensor.matmul(out=z1p[:], lhsT=Ws_s["W1"][:], rhs=bn1[:], start=True, stop=True)
            zs1 = pp5.tile([16, G], f32, tag="zs1")
            nc.scalar.activation(out=zs1[:], in_=z1p[:], func=AF.Relu, bias=cols_s["bW1"][:], scale=1.0)
            bn2a = pp5.tile([16, G], f32, tag="bn2a")
            bn_t(zs1[:], 16, cols_s["gl2a"][:], cols_s["bl2a"][:], bn2a[:])
            bn2b = pp5.tile([16, G], f32, tag="bn2b")
            bn_t(xeT[:], 16, cols_s["gl2b"][:], cols_s["bl2b"][:], bn2b[:])
            z2p = ps5.tile([16, G], f32, tag="z2p")
            nc.tensor.matmul(out=z2p[:], lhsT=Ws_s["W2a"][:], rhs=bn2a[:], start=True, stop=False)
            nc.tensor.matmul(out=z2p[:], lhsT=Ws_s["W2b"][:], rhs=bn2b[:], start=False, stop=True)
            zs2 = pp5.tile([16, G], f32, tag="zs2")
            nc.scalar.activation(out=zs2[:], in_=z2p[:], func=AF.Relu, bias=cols_s["bW2"][:], scale=1.0)
            bn3a = pp5.tile([16, G], f32, tag="bn3a")
            bn_t(zs2[:], 16, cols_s["gl3a"][:], cols_s["bl3a"][:], bn3a[:])
            bn3b = pp5.tile([16, G], f32, tag="bn3b")
            bn_t(xeT[:], 16, cols_s["gl3b"][:], cols_s["bl3b"][:], bn3b[:])
            z3p = ps5.tile([16, G], f32, tag="z3p")
            nc.tensor.matmul(out=z3p[:], lhsT=Ws_s["W3a"][:], rhs=bn3a[:], start=True, stop=False)
            nc.tensor.matmul(out=z3p[:], lhsT=Ws_s["W3b"][:], rhs=bn3b[:], start=False, stop=True)
            z3 = pp5.tile([16, G], f32, tag="z3")
            nc.scalar.activation(out=z3[:], in_=z3p[:], func=AF.Relu, bias=cols_s["bW3"][:], scale=1.0)
            ofp = ps5.tile([1, G], f32, tag="ofp")
            nc.tensor.matmul(out=ofp[:], lhsT=Ws_s["Wf"][:], rhs=z3[:], start=True, stop=True)
            ofs = pp5.tile([1, G], f32, tag="ofs")
            nc.vector.tensor_scalar(
                out=ofs[:], in0=ofp[:], scalar1=cols_s["bWf"][:], scalar2=None, op0=ALU.add
            )
            nc.sync.dma_start(out=out_t.ap(), in_=ofs[:])

    nc.compile()
    return nc


def run(inputs, n_cores=8, G=256, cfg_overrides=None, trace=False, enable_asserts=False):
    from concourse.bass_utils import run_bass_kernel_spmd

    in_maps, cfg = build_host_data(
        inputs["x"], inputs["edge_index"], inputs["batch"], n_cores, G
    )
    if cfg_overrides:
        cfg.update(cfg_overrides)
    add_weights(in_maps, inputs)
    nc = build_program(cfg, enable_asserts=enable_asserts)
    res = run_bass_kernel_spmd(nc, in_maps, core_ids=list(range(n_cores)), trace=trace)
    out = res.results[0]["out"].reshape(G, 1)
    return out, res, cfg


def kernel(**inputs):
    """Full inputs -> full [256, 1] output. Shards internally across 8 cores."""
    out, _, _ = run(inputs, n_cores=8, G=256)
    return np.asarray(out, np.float32)
